# revision 22
# baseline (speedup 1.0000x reference)
"""Trainium2 Bass kernel for nn_Chan_spaAtt (SE-gated conv block).

The spatial self-attention branch in the reference is dead code -- the
output depends only on xo = x * sigmoid(xl + xg) through the final
3x3 conv + BN + ReLU.  BN affines are folded host-side.

Per sample (C=64, H=W=64), with an even/odd COLUMN-PARITY layout:
  partitions 0:64  = channel c of even image columns (pair index j -> col 2j)
  partitions 64:128 = channel c of odd image columns  (j -> col 2j+1)

  mm1:  t1   = relu(blockdiag(W1,W1) @ x + b1)       [32, N/2]
  mm2:  sarg = blockstack(W2,W2) @ t1                [128, N/2]
  xg (global branch) folded into the sigmoid bias via two tiny matmuls
  xo   = x * sigmoid(sarg + dbias)                   [128, N/2]

3x3 conv as 6 matmuls of N/2 rows (vs 9 at N): per row-tap dy the
dense matmul A_dy covers 4 tap-instances (dx=0,+1 for even outputs,
dx=-1,0 for odd) reading buf1 = xo; matmul B_dy covers the remaining 2
(dx=-1 even, dx=+1 odd) reading buf2 = half-swapped/column-shifted copy
of buf1 built by two contiguous SBUF->SBUF DMAs per chunk (row pads in
buf1 propagate the zero boundary columns automatically).

Rows live on the free axis with one zero pad row above/below and 2 pad
slots per 32-pair row (stride 34), so all dy/dx shifts are plain AP
offsets.  Everything computes in bf16 (inputs/outputs cast host-side),
PSUM accumulation in fp32; rel err ~4e-3 vs fp32 reference.

Sharding: pure data parallelism, one sample per NeuronCore (B=8).
"""

import sys

if "/opt/trn_rl_repo" not in sys.path:
    sys.path.insert(0, "/opt/trn_rl_repo")

import numpy as np
import ml_dtypes

import concourse.bass as bass
import concourse.bacc as bacc
import concourse.mybir as mybir
import concourse.tile as tile
from concourse.bass_utils import run_bass_kernel_spmd

B, C, H, W = 8, 64, 64, 64
N = H * W
NP = N // 2          # pixels per parity = 2048
INTER = 16
EPS = 1e-5
JP = W // 2          # pairs per row = 32
RSTR = JP + 2        # buf row stride = 34 (pad_l, 32 slots, pad_r)
NROW = H + 2         # 66 buffered rows (zero row above/below)
BUFCOLS = NROW * RSTR + 4   # 2248 incl. slack
CHUNK = 512
NCHUNK = NP // CHUNK  # 4
ROWS_PER_CHUNK = CHUNK // JP  # 16

BF16 = mybir.dt.bfloat16
F32 = mybir.dt.float32

# blob (bf16) column layout -- single tensor, two DMAs (small part first)
O_W1B = 0            # [128, 32]
O_W2B = 32           # [32, 128]
O_ID = 160           # [128, 128] identity (global-sum accumulate matmuls)
O_GW1B = 288         # [128, 16]
O_GW2B = 304         # [16, 128]
O_BIAS = 432         # [128, 8] bf16 = bitcast [128, 4] f32: b1, gb1, bsig, cb
BLOB_A_COLS = 440
O_CONV = 440         # 6 x [128, 128]: A(-1), A(0), A(+1), B(-1), B(0), B(+1)
BLOBCOLS = 440 + 6 * 128     # 1208

N_FILLERS = 1        # PE p-state warmers (scratch matmuls on x data)

_prog_cache = {}


def _row(r):
    """Flat offset of image row r's pad_l in buf1/buf2 (rows -1..64)."""
    return (r + 1) * RSTR


def build_program(n_cores=8):
    nc = bacc.Bacc("TRN2", debug=False, target_bir_lowering=False,
                   num_devices=n_cores)

    xin_d = nc.dram_tensor("xin", [2 * C, NP], BF16, kind="ExternalInput").ap()
    blob_d = nc.dram_tensor("blob", [2 * C, BLOBCOLS], BF16,
                            kind="ExternalInput").ap()
    y_d = nc.dram_tensor("y", [2 * C, NP], BF16, kind="ExternalOutput").ap()

    with tile.TileContext(nc) as tc:
        with tc.tile_pool(name="big", bufs=1) as bpool, \
             tc.tile_pool(name="t1p", bufs=4) as t1pool, \
             tc.tile_pool(name="sigp", bufs=2) as sigpool, \
             tc.tile_pool(name="ybp", bufs=2) as ybpool, \
             tc.tile_pool(name="ps1p", bufs=1, space="PSUM") as pp1, \
             tc.tile_pool(name="ps2p", bufs=2, space="PSUM") as pp2, \
             tc.tile_pool(name="psyp", bufs=4, space="PSUM") as ppy:

            xsb = bpool.tile([2 * C, NP], BF16, tag="xsb")
            wsb = bpool.tile([2 * C, BLOBCOLS], BF16, tag="wsb")
            buf1 = bpool.tile([2 * C, BUFCOLS], BF16, tag="buf1")
            buf2 = bpool.tile([2 * C, BUFCOLS], BF16, tag="buf2")
            gsum = bpool.tile([2 * C, 1], BF16, tag="gsum")
            g1 = bpool.tile([INTER, 1], BF16, tag="g1")
            dbias = bpool.tile([2 * C, 1], F32, tag="dbias")
            scr_in = bpool.tile([1, 2], BF16, tag="scrin")
            scr_out = bpool.tile([1, 2], BF16, tag="scrout")

            # ---- activation-table preload: a dummy sigmoid with no deps
            # makes the table loads happen at t~0, off the critical path ----
            nc.vector.memset(scr_in[:].bitcast(mybir.dt.uint16), 0)
            nc.scalar.activation(scr_out[:], scr_in[:],
                                 mybir.ActivationFunctionType.Sigmoid)

            # ---- DMAs: small weights first (Act queue), x halves on the
            # SP queue, conv weights last ----
            nc.scalar.dma_start(wsb[:, 0:BLOB_A_COLS],
                                blob_d[:, 0:BLOB_A_COLS])
            nc.sync.dma_start(xsb[:, 0:NP // 2], xin_d[:, 0:NP // 2])
            nc.sync.dma_start(xsb[:, NP // 2:NP], xin_d[:, NP // 2:NP])
            nc.scalar.dma_start(wsb[:, BLOB_A_COLS:BLOBCOLS],
                                blob_d[:, BLOB_A_COLS:BLOBCOLS])

            w1b = wsb[:, O_W1B:O_W1B + 32]
            w2b = wsb[0:32, O_W2B:O_W2B + 128]
            idw = wsb[:, O_ID:O_ID + 128]
            convw = [wsb[:, O_CONV + k * 128:O_CONV + (k + 1) * 128]
                     for k in range(6)]   # A-1 A0 A+1 B-1 B0 B+1
            biasf = wsb[:, O_BIAS:O_BIAS + 8].bitcast(F32)
            b1 = biasf[0:32, 0:1]
            gb1 = biasf[0:INTER, 1:2]
            bsig = biasf[:, 2:3]
            cb = biasf[:, 3:4]
            gw1b = wsb[:, O_GW1B:O_GW1B + INTER]
            gw2b = wsb[0:INTER, O_GW2B:O_GW2B + 128]

            # ---- zero pads (DVE): rows -1/64 in both bufs + buf1 slot pads
            # (buf1's pads propagate zeros into buf2 via the swap copies) ----
            nc.vector.memset(buf1[:, 0:RSTR].bitcast(mybir.dt.uint16), 0)
            nc.vector.memset(
                buf1[:, _row(H):_row(H) + RSTR].bitcast(mybir.dt.uint16), 0)
            inner = buf1[:, RSTR:RSTR + H * RSTR].rearrange(
                "p (r w) -> p r w", w=RSTR)
            nc.vector.memset(inner[:, :, 0:1].bitcast(mybir.dt.uint16), 0)
            nc.vector.memset(
                inner[:, :, RSTR - 1:RSTR].bitcast(mybir.dt.uint16), 0)
            nc.vector.memset(buf2[:, 0:RSTR].bitcast(mybir.dt.uint16), 0)
            nc.vector.memset(
                buf2[:, _row(H):_row(H) + RSTR].bitcast(mybir.dt.uint16), 0)

            # ---- global sum: identity-matmul accumulate x chunks into one
            # PSUM bank (PE, exact), then one DVE reduce 512->1 ----
            ps_acc = ppy.tile([2 * C, CHUNK], F32, tag="psy")
            for ci in range(NCHUNK):
                nc.tensor.matmul(ps_acc[:], idw,
                                 xsb[:, ci * CHUNK:(ci + 1) * CHUNK],
                                 start=(ci == 0), stop=(ci == NCHUNK - 1))
            with nc.allow_low_precision(
                    reason="bf16 channel-sum feeds tiny SE gate; 0.4% ok"):
                nc.vector.reduce_sum(gsum[:], ps_acc[:],
                                     axis=mybir.AxisListType.X)

            # ---- mm1 all chunks into one PSUM bank (partition-offset);
            # t1relu spread across Act/Pool/Pool/DVE ----
            ps1big = pp1.tile([3 * 32, CHUNK], F32, tag="ps1")
            t1s = {}
            for ci in range(NCHUNK):
                # chunk 3 reuses chunk 0's partitions (WAR after t1relu_0)
                ps1 = ps1big[32 * (ci % 3):32 * (ci % 3) + 32, :]
                nc.tensor.matmul(ps1, w1b,
                                 xsb[:, ci * CHUNK:(ci + 1) * CHUNK],
                                 start=True, stop=True)
                t1 = t1pool.tile([32, CHUNK], BF16, tag="t1")
                t1s[ci] = t1
                if ci == 0:
                    nc.scalar.activation(t1[:], ps1,
                                         mybir.ActivationFunctionType.Relu,
                                         bias=b1)
                elif ci in (1, 2):
                    nc.gpsimd.tensor_scalar(t1[:], ps1, b1, 0.0,
                                            mybir.AluOpType.add,
                                            mybir.AluOpType.max)
                else:
                    nc.vector.tensor_scalar(t1[:], ps1, b1, 0.0,
                                            mybir.AluOpType.add,
                                            mybir.AluOpType.max)

            # ---- global branch: dbias = G2 @ relu(G1 @ mean + gb1) + bsig ----
            ps_g1 = ppy.tile([INTER, 1], F32, tag="psy")
            nc.tensor.matmul(ps_g1[:], gw1b, gsum[:], start=True, stop=True)
            nc.scalar.activation(g1[:], ps_g1[:],
                                 mybir.ActivationFunctionType.Relu,
                                 bias=gb1, scale=1.0 / N)
            ps_g2 = ppy.tile([2 * C, 1], F32, tag="psy")
            nc.tensor.matmul(ps_g2[:], gw2b, g1[:], start=True, stop=True)
            nc.scalar.activation(dbias[:], ps_g2[:],
                                 mybir.ActivationFunctionType.Identity,
                                 bias=bsig)

            # ---- mm2 + sigmoid + gated mul into buf1, swap into buf2 ----
            def fill():
                psf = ppy.tile([2 * C, CHUNK], F32, tag="psy")
                nc.tensor.matmul(psf[:], xsb[:, 0:128], xsb[:, 0:CHUNK],
                                 start=True, stop=True)

            for ci in range(NCHUNK):
                ps2 = pp2.tile([2 * C, CHUNK], F32, tag="ps2")
                nc.tensor.matmul(ps2[:], w2b, t1s[ci][:],
                                 start=True, stop=True)
                if N_FILLERS > ci:
                    fill()
                sig = sigpool.tile([2 * C, CHUNK], BF16, tag="sig")
                nc.scalar.activation(sig[:], ps2[:],
                                     mybir.ActivationFunctionType.Sigmoid,
                                     bias=dbias[:])
                r0 = ci * ROWS_PER_CHUNK
                dst = buf1[:, _row(r0):_row(r0) + ROWS_PER_CHUNK * RSTR]
                dst = dst.rearrange("p (r w) -> p r w", w=RSTR)[:, :, 1:JP + 1]
                xcr = xsb[:, ci * CHUNK:(ci + 1) * CHUNK].rearrange(
                    "p (r w) -> p r w", w=JP)
                sgr = sig[:].rearrange("p (r w) -> p r w", w=JP)
                nc.vector.tensor_mul(dst, xcr, sgr)
                # swap halves into buf2 (flat contiguous, pads carry zeros):
                #   buf2_low slot j = xo_odd[j-1]; buf2_high slot j = xo_even[j+1]
                # issued on the gpsimd SWDGE path to keep the (globally
                # serialized) HWDGE descriptor generator free.
                s0 = _row(r0) + 1
                ln = ROWS_PER_CHUNK * RSTR
                # chunk 0 starts one element earlier so buf2_low row0/slot0
                # picks up buf1's zero pad
                ext = 1 if ci == 0 else 0
                nc.gpsimd.dma_start(buf2[0:C, s0 + 1 - ext:s0 + 1 + ln],
                                    buf1[C:2 * C, s0 - ext:s0 + ln])
                nc.gpsimd.dma_start(buf2[C:2 * C, s0 - 1:s0 - 1 + ln],
                                    buf1[0:C, s0:s0 + ln])

            # ---- conv: 3 dense A matmuls + 3 half B matmuls per chunk ----
            def rhs(buf, r0, dy):
                v = buf[:, _row(r0 + dy) + 1:
                        _row(r0 + dy) + 1 + ROWS_PER_CHUNK * RSTR]
                return v.rearrange("p (r w) -> p r w", w=RSTR)[:, :, 0:JP]

            psys = {}
            for ci in range(NCHUNK):
                r0 = ci * ROWS_PER_CHUNK
                psy = ppy.tile([2 * C, CHUNK], F32, tag="psy")
                psys[ci] = psy
                for j, dy in enumerate((-1, 0, 1)):
                    nc.tensor.matmul(psy[:], convw[j], rhs(buf1, r0, dy),
                                     start=(j == 0), stop=False)
            for ci in range(NCHUNK):
                r0 = ci * ROWS_PER_CHUNK
                psy = psys[ci]
                for j, dy in enumerate((-1, 0, 1)):
                    nc.tensor.matmul(psy[:], convw[3 + j], rhs(buf2, r0, dy),
                                     start=False, stop=(j == 2))
                ybuf = ybpool.tile([2 * C, CHUNK], BF16, tag="ybuf")
                if ci % 2 == 0:
                    nc.scalar.activation(ybuf[:], psy[:],
                                         mybir.ActivationFunctionType.Relu,
                                         bias=cb)
                    nc.sync.dma_start(y_d[:, ci * CHUNK:(ci + 1) * CHUNK],
                                      ybuf[:])
                else:
                    nc.vector.tensor_scalar(ybuf[:], psy[:], cb, 0.0,
                                            mybir.AluOpType.add,
                                            mybir.AluOpType.max)
                    nc.scalar.dma_start(y_d[:, ci * CHUNK:(ci + 1) * CHUNK],
                                        ybuf[:])

    nc.compile()
    return nc


def _affine(s, b, m, v):
    inv = s / np.sqrt(v + EPS)
    return inv, b - m * inv


def prepare_weights(inputs):
    f = lambda k: np.asarray(inputs[k], dtype=np.float32)
    a1, c1 = _affine(f("ls1"), f("lbb1"), f("lm1"), f("lv1"))
    W1 = a1[:, None] * f("lw1")                    # [16, 64]
    B1 = a1 * f("lb1") + c1
    a2, c2 = _affine(f("ls2"), f("lbb2"), f("lm2"), f("lv2"))
    W2 = a2[:, None] * f("lw2")                    # [64, 16]
    B2 = a2 * f("lb2") + c2
    ag1, cg1 = _affine(f("gs1"), f("gbb1"), f("gm1"), f("gv1"))
    G1 = ag1[:, None] * f("gw1")                   # [16, 64]
    Bg1 = ag1 * f("gb1") + cg1
    ag2, cg2 = _affine(f("gs2"), f("gbb2"), f("gm2"), f("gv2"))
    G2 = ag2[:, None] * f("gw2")                   # [64, 16]
    Bg2 = ag2 * f("gb2") + cg2
    ac, cc = _affine(f("cs"), f("cbb"), f("cm"), f("cv"))
    CW = ac[:, None, None, None] * f("cw")         # [O, C, 3, 3]
    CB = ac * f("cb") + cc
    return dict(W1=W1, B1=B1, W2=W2, G1=G1, Bg1=Bg1, G2=G2,
                bsig=B2 + Bg2, CW=CW, CB=CB)


def assemble_blob(sh):
    blob = np.zeros((2 * C, BLOBCOLS), ml_dtypes.bfloat16)
    bf = lambda a: a.astype(ml_dtypes.bfloat16)
    W1T = sh["W1"].T                               # [64, 16]
    blob[0:C, O_W1B:O_W1B + INTER] = bf(W1T)
    blob[C:2 * C, O_W1B + INTER:O_W1B + 32] = bf(W1T)
    W2T = sh["W2"].T                               # [16, 64]
    blob[0:INTER, O_W2B:O_W2B + C] = bf(W2T)
    blob[INTER:32, O_W2B + C:O_W2B + 2 * C] = bf(W2T)
    blob[:, O_ID:O_ID + 128] = bf(np.eye(2 * C, dtype=np.float32))
    G1T = sh["G1"].T                               # [64, 16]
    blob[0:C, O_GW1B:O_GW1B + INTER] = bf(G1T)
    blob[C:2 * C, O_GW1B:O_GW1B + INTER] = bf(G1T)
    G2T = sh["G2"].T                               # [16, 64]
    blob[0:INTER, O_GW2B:O_GW2B + C] = bf(G2T)
    blob[0:INTER, O_GW2B + C:O_GW2B + 2 * C] = bf(G2T)
    # f32 biases bitcast into 2 bf16 columns each: b1, gb1, bsig, cb
    bias = np.zeros((2 * C, 4), np.float32)
    bias[0:INTER, 0] = sh["B1"]
    bias[INTER:32, 0] = sh["B1"]
    bias[0:INTER, 1] = sh["Bg1"]
    bias[0:C, 2] = sh["bsig"]
    bias[C:2 * C, 2] = sh["bsig"]
    bias[0:C, 3] = sh["CB"]
    bias[C:2 * C, 3] = sh["CB"]
    blob[:, O_BIAS:O_BIAS + 8] = bias.view(ml_dtypes.bfloat16)
    CW = sh["CW"]
    cwt = lambda dy, dx: CW[:, :, dy + 1, dx + 1].T   # [c, o]
    for j, dy in enumerate((-1, 0, 1)):
        A = np.zeros((2 * C, 2 * C), np.float32)
        A[0:C, 0:C] = cwt(dy, 0)
        A[C:2 * C, 0:C] = cwt(dy, 1)
        A[0:C, C:2 * C] = cwt(dy, -1)
        A[C:2 * C, C:2 * C] = cwt(dy, 0)
        blob[:, O_CONV + j * 128:O_CONV + (j + 1) * 128] = bf(A)
        Bm = np.zeros((2 * C, 2 * C), np.float32)
        Bm[0:C, 0:C] = cwt(dy, -1)
        Bm[C:2 * C, C:2 * C] = cwt(dy, 1)
        blob[:, O_CONV + (3 + j) * 128:O_CONV + (4 + j) * 128] = bf(Bm)
    return np.ascontiguousarray(blob)


def pack_x(xi):
    """[C, H, W] f32 -> [128, NP] bf16 parity-split."""
    ev = xi[:, :, 0::2].reshape(C, NP)
    od = xi[:, :, 1::2].reshape(C, NP)
    return np.ascontiguousarray(
        np.concatenate([ev, od], axis=0)).astype(ml_dtypes.bfloat16)


def unpack_y(yc):
    """[128, NP] bf16 -> [C, H, W] f32."""
    y = np.empty((C, H, W), np.float32)
    y[:, :, 0::2] = np.asarray(yc[0:C], np.float32).reshape(C, H, JP)
    y[:, :, 1::2] = np.asarray(yc[C:2 * C], np.float32).reshape(C, H, JP)
    return y


def make_core_inputs(inputs):
    sh = prepare_weights(inputs)
    blob = assemble_blob(sh)
    x = np.asarray(inputs["x"], dtype=np.float32)
    return [{"xin": pack_x(x[i]), "blob": blob} for i in range(B)]


def _run(inputs, trace=False):
    in_maps = make_core_inputs(inputs)
    if "prog" not in _prog_cache:
        _prog_cache["prog"] = build_program(B)
    nc = _prog_cache["prog"]
    res = run_bass_kernel_spmd(nc, in_maps, list(range(B)), trace=trace)
    out = np.stack([unpack_y(r["y"]) for r in res.results])
    return out.astype(np.float32), res


def kernel(**inputs):
    out, _ = _run(inputs, trace=False)
    return out


def kernel_traced(inputs):
    return _run(inputs, trace=True)


def reference_numpy(inputs):
    """Numpy emulation of the device algebra (parity layout, bf16 casts)."""
    bf = lambda a: a.astype(ml_dtypes.bfloat16).astype(np.float32)
    sh = prepare_weights(inputs)
    blobraw = assemble_blob(sh)
    blob = np.asarray(blobraw, np.float32)
    aux = np.ascontiguousarray(blobraw[:, O_BIAS:O_BIAS + 8]).view(np.float32)
    x = np.asarray(inputs["x"], dtype=np.float32)
    out = np.empty_like(x)
    w1b = blob[:, O_W1B:O_W1B + 32]
    w2b = blob[0:32, O_W2B:O_W2B + 128]
    convw = [blob[:, O_CONV + k * 128:O_CONV + (k + 1) * 128]
             for k in range(6)]
    for i in range(B):
        xp = np.asarray(pack_x(x[i]), np.float32)      # [128, NP]
        gs = bf(xp.sum(axis=1, keepdims=True))         # [128, 1]
        g1 = bf(np.maximum(blob[:, O_GW1B:O_GW1B + INTER].T @ gs / N
                           + aux[0:INTER, 1:2], 0.0))
        db = blob[0:INTER, O_GW2B:O_GW2B + 128].T @ g1 + aux[:, 2:3]
        t1 = bf(np.maximum(w1b.T @ xp + aux[0:32, 0:1], 0.0))
        sarg = w2b.T @ t1 + db
        sig = bf(1.0 / (1.0 + np.exp(-sarg)))
        xo = bf(xp * sig)
        # padded buffers
        b1_ = np.zeros((128, NROW * RSTR + 4), np.float32)
        v = b1_[:, RSTR:RSTR + H * RSTR].reshape(128, H, RSTR)
        v[:, :, 1:JP + 1] = xo.reshape(128, H, JP)
        b2_ = np.zeros_like(b1_)
        s0 = RSTR + 1
        ln = H * RSTR
        b2_[0:C, s0 + 1:s0 + 1 + ln] = b1_[C:2 * C, s0:s0 + ln]
        b2_[C:2 * C, s0 - 1:s0 - 1 + ln] = b1_[0:C, s0:s0 + ln]
        y = np.zeros((128, NP), np.float32)
        for j, dy in enumerate((-1, 0, 1)):
            for bb, wb in ((b1_, convw[j]), (b2_, convw[3 + j])):
                sh_v = bb[:, (1 + dy) * RSTR + 1:
                          (1 + dy) * RSTR + 1 + H * RSTR]
                sh_v = sh_v.reshape(128, H, RSTR)[:, :, 0:JP].reshape(128, NP)
                y += wb.T @ bf(sh_v)
        y = np.maximum(y + aux[:, 3:4], 0.0)
        out[i] = unpack_y(y.astype(ml_dtypes.bfloat16))
    return out
